# revision 20
# baseline (speedup 1.0000x reference)
"""CopyGenerator on 8 TRN2 NeuronCores.

Strategy: tensor-parallel split of the 50257-wide generator vocab across the
8 cores (6400 padded columns each).  Each core:
  - holds its W_gen shard (fp8 e4m3, host-scaled by 64 and pre-transposed)
    and hidden^T (fp8) resident in SBUF,
  - computes logits*64 = hidden @ (64*W_shard).T with fp8 DoubleRow matmuls
    (256-deep contraction per instruction, fp32 PSUM accum),
  - applies exp(psum/64) on the Scalar engine over 2048-wide PSUM
    superchunks (accum_out gives the row partial sums),
  - all-gathers softmax partial denominators across cores in batches of
    2-4 row tiles (5 collectives total, overlapped with later tiles),
  - scales exp by (1 - p_copy)/denom on the Vector engine (bf16 4x mode)
    and writes its bf16 output shard,
  - computes the (tiny) copy-attention path redundantly at the end.
PAD column and vocab-padding columns are handled by zeroing those W rows on
the host (=> logit 0, exp 1) and subtracting the per-core masked-column
count from the partial denominator; the host zeroes the PAD output column.
p_copy = sigmoid(hidden @ W_copy.T + b_copy) is a 2 MFLOP per-row scalar;
it is computed host-side in f32 and fed in as two small tensors.

kernel(**inputs) takes the full unsharded inputs and returns the full
[2048, 50321] float32 output.
"""

import os
import sys

for _p in ("/opt/trn_rl_repo", "/opt/trn_rl_repo/concourse"):
    if _p not in sys.path:
        sys.path.insert(0, _p)

from contextlib import ExitStack

import ml_dtypes
import numpy as np

import concourse.bass as bass
import concourse.mybir as mybir
import concourse.tile as tile
from concourse import bacc
from concourse.bass_utils import run_bass_kernel_spmd

# ---- problem constants (hardcoded per the self-contained-kernel contract) ----
N, D = 2048, 1024                 # tlen*batch rows, hidden dim
TLEN, BATCH, SLEN, CVOCAB = 64, 32, 128, 64
VOCAB = 50257
PAD_IDX = 0
NCORES = 8
VS = 6400                         # per-core padded vocab shard width
VPAD = VS * NCORES                # 51200
DT = D // 128                     # 8 contraction tiles
NT = N // 128                     # 16 row tiles
WSCALE = 64.0                     # host pre-scale on W (fp8 subnormal escape)

# matmul/exp chunks: [(col offset, width)]; each is one 2-bank PSUM tile
# (bufs=4 -> ring depth 4, so a chunk's matmuls never wait on its own
# activation draining -- ~4.6us of slack per chunk)
SC = [(q, 1024) for q in range(0, 6144, 1024)] + [(6144, 256)]
# scale/store chunks (read exp from SBUF; wider to amortize DVE/DMA fixed
# costs -- independent of the PSUM chunking above)
SCOUT = [(0, 2048), (2048, 2048), (4096, 2048), (6144, 256)]
# denominator all-gather batching: [(first tile, n tiles)]; first group is
# small so the first collective (15us fixed latency) completes early, and the
# last two are single tiles so only one tile's scale+store sits in the tail.
GROUPS = [(0, 2), (2, 4), (6, 4), (10, 4), (14, 2)]

BF16 = ml_dtypes.bfloat16
FP8 = ml_dtypes.float8_e4m3
F32 = mybir.dt.float32
BF16_T = mybir.dt.bfloat16
FP8_T = mybir.dt.float8e4
DR = mybir.MatmulPerfMode.DoubleRow

LAST_RESULTS = None               # BassKernelResults of the most recent run
_NC_CACHE = {}


def _build(use_bgen: bool):
    nc = bacc.Bacc("TRN2", target_bir_lowering=False, debug=False,
                   num_devices=NCORES)

    wt = nc.dram_tensor("wt", [128, DT * VS], FP8_T, kind="ExternalInput").ap()
    ht = nc.dram_tensor("ht", [128, DT * N], FP8_T, kind="ExternalInput").ap()
    attn_r = nc.dram_tensor("attn_r", [128, BATCH * TLEN], BF16_T,
                            kind="ExternalInput").ap()
    smap = nc.dram_tensor("smap", [128, BATCH * CVOCAB], BF16_T,
                          kind="ExternalInput").ap()
    pce = nc.dram_tensor("pce", [TLEN, BATCH * CVOCAB], F32,
                         kind="ExternalInput").ap()
    ompc = nc.dram_tensor("ompc", [128, NT], F32, kind="ExternalInput").ap()
    mneg = nc.dram_tensor("mneg", [1, 1], F32, kind="ExternalInput").ap()
    if use_bgen:
        bg = nc.dram_tensor("bg", [1, VS], BF16_T, kind="ExternalInput").ap()
    out_main = nc.dram_tensor("out_main", [N, VS], BF16_T,
                              kind="ExternalOutput").ap()
    out_copy = nc.dram_tensor("out_copy", [N, CVOCAB], F32,
                              kind="ExternalOutput").ap()

    with tile.TileContext(nc) as tc, ExitStack() as ctx:
        singles = ctx.enter_context(tc.tile_pool(name="singles", bufs=1))
        dram = ctx.enter_context(tc.tile_pool(name="dram", bufs=1, space="DRAM"))

        # ---- resident inputs ----
        # interleave hidden^T and first-superchunk W by dp-pair so tile 0's
        # first matmuls start after ~3us of DMA instead of the full load
        ht_sb = singles.tile([128, DT, N], FP8_T)
        wt_sb = singles.tile([128, DT, VS], FP8_T)
        ht3 = ht.rearrange("p (d n) -> p d n", d=DT)
        for dp in range(DT // 2):
            nc.sync.dma_start(out=ht_sb[:, 2 * dp:2 * dp + 2, :],
                              in_=ht3[:, 2 * dp:2 * dp + 2, :])
            for d in (2 * dp, 2 * dp + 1):
                nc.gpsimd.dma_start(out=wt_sb[:, d, 0:1024],
                                    in_=wt[:, d * VS:d * VS + 1024])
        for lo, hi in ((1024, 3072), (3072, 6400)):
            for d in range(DT):
                nc.gpsimd.dma_start(out=wt_sb[:, d, lo:hi],
                                    in_=wt[:, d * VS + lo:d * VS + hi])
        ompc_sb = singles.tile([128, NT], F32)
        nc.sync.dma_start(out=ompc_sb, in_=ompc)
        mneg_sb = singles.tile([128, 1], F32)
        nc.sync.dma_start(out=mneg_sb, in_=mneg.to_broadcast((128, 1)))
        # copy-path inputs are only needed at the end; keep them last in queue
        attn_sb = singles.tile([128, BATCH * TLEN], BF16_T)
        nc.gpsimd.dma_start(out=attn_sb, in_=attn_r)
        sm_sb = singles.tile([128, BATCH * CVOCAB], BF16_T)
        nc.gpsimd.dma_start(out=sm_sb, in_=smap)
        pce_sb = singles.tile([TLEN, BATCH * CVOCAB], F32)
        nc.gpsimd.dma_start(out=pce_sb, in_=pce)
        if use_bgen:
            bg_sb = singles.tile([1, VS], BF16_T)
            nc.sync.dma_start(out=bg_sb, in_=bg)
            ones_sb = singles.tile([1, N], BF16_T)
            nc.vector.memset(ones_sb, 1.0)

        expp = ctx.enter_context(tc.tile_pool(name="expp", bufs=7))
        accp = ctx.enter_context(tc.tile_pool(name="accp", bufs=3))
        ccp = ctx.enter_context(tc.tile_pool(name="ccp", bufs=2))
        smallp = ctx.enter_context(tc.tile_pool(name="small", bufs=2))
        ostp = ctx.enter_context(tc.tile_pool(name="ostp", bufs=4))
        psp = ctx.enter_context(tc.tile_pool(name="ps", bufs=4, space="PSUM"))

        # ---- main loop: 16 row tiles in 5 denominator groups ----
        # The scale/store block for group k-1 is emitted AFTER group k's
        # collective is issued: the in-order Vector queue then never blocks
        # on a collective that hasn't had a full group of compute to hide in.
        exps = {}
        pending = []                  # [(t0, G, ccout)] awaiting scale block

        def emit_scales(t0, G, ccout):
            parts = smallp.tile([128, G, NCORES], F32, tag="parts",
                                padded_shape=[128, 4, NCORES])
            nc.scalar.dma_start(
                out=parts,
                in_=ccout.rearrange("(r p g) -> p g r", p=128, g=G))
            den = smallp.tile([128, G], F32, tag="den",
                              padded_shape=[128, 4])
            nc.vector.reduce_sum(den, parts, axis=mybir.AxisListType.X)
            rden = smallp.tile([128, G], F32, tag="rden",
                               padded_shape=[128, 4])
            nc.vector.reciprocal(rden, den)
            fs = smallp.tile([128, G], F32, tag="fs", padded_shape=[128, 4])
            nc.vector.tensor_mul(fs, rden, ompc_sb[:, t0:t0 + G])
            for j in range(t0, t0 + G):
                for c0, cw in SCOUT:
                    ost = ostp.tile([128, cw], BF16_T, tag="ost",
                                    padded_shape=[128, 2048])
                    nc.vector.tensor_scalar_mul(ost, exps[j][:, c0:c0 + cw],
                                                fs[:, j - t0:j - t0 + 1])
                    nc.sync.dma_start(
                        out=out_main[j * 128:(j + 1) * 128, c0:c0 + cw],
                        in_=ost)
                del exps[j]

        def emit_copy_path():
            # per-batch [64t,128s] @ [128s,64c], x p_copy; tiny -- emitted
            # mid-kernel so none of its work lands in the tail
            oc_flat = out_copy.rearrange("(t b) c -> t (b c)", b=BATCH)
            BB = 8                               # batches per psum tile
            for g in range(BATCH // BB):
                cp = psp.tile([TLEN, BB * CVOCAB], F32, tag="psm",
                              padded_shape=[128, 1024])
                for bb in range(BB):
                    b = g * BB + bb
                    nc.tensor.matmul(
                        cp[:, bb * CVOCAB:(bb + 1) * CVOCAB],
                        lhsT=attn_sb[:, b * TLEN:(b + 1) * TLEN],
                        rhs=sm_sb[:, b * CVOCAB:(b + 1) * CVOCAB],
                        start=True, stop=True,
                    )
                oc = ostp.tile([TLEN, BB * CVOCAB], F32, tag="oc", bufs=2)
                nc.vector.tensor_mul(
                    oc, cp, pce_sb[:, g * BB * CVOCAB:(g + 1) * BB * CVOCAB])
                nc.sync.dma_start(
                    out=oc_flat[:, g * BB * CVOCAB:(g + 1) * BB * CVOCAB],
                    in_=oc)

        for gi, (t0, G) in enumerate(GROUPS):
            ccin = ccp.tile([128, G], F32, tag="ccin",
                            padded_shape=[128, 4])
            for j in range(t0, t0 + G):
                n0 = j * 128
                exp_sb = expp.tile([128, VS], BF16_T, tag="exp")
                exps[j] = exp_sb
                acc4 = accp.tile([128, len(SC)], F32, tag="acc")
                for ci, (c0, cw) in enumerate(SC):
                    psm = psp.tile([128, cw], F32, tag="psm",
                                   padded_shape=[128, 1024])
                    for dp in range(DT // 2):
                        for q in range(0, cw, 512):
                            qw = min(512, cw - q)
                            nc.tensor.matmul(
                                psm[:, q:q + qw],
                                lhsT=ht_sb[:, 2 * dp:2 * dp + 2, n0:n0 + 128],
                                rhs=wt_sb[:, 2 * dp:2 * dp + 2,
                                          c0 + q:c0 + q + qw],
                                start=(dp == 0),
                                stop=(dp == DT // 2 - 1) and not use_bgen,
                                perf_mode=DR,
                            )
                    if use_bgen:
                        nq = [q for q in range(0, cw, 512)]
                        for qi, q in enumerate(nq):
                            qw = min(512, cw - q)
                            nc.tensor.matmul(
                                psm[:, q:q + qw],
                                lhsT=ones_sb[:, n0:n0 + 128],
                                rhs=bg_sb[:, c0 + q:c0 + q + qw],
                                start=False, stop=(qi == len(nq) - 1),
                                skip_group_check=True,
                            )
                    nc.scalar.activation(exp_sb[:, c0:c0 + cw], psm[:, 0:cw],
                                         mybir.ActivationFunctionType.Exp,
                                         scale=1.0 / WSCALE,
                                         accum_out=acc4[:, ci:ci + 1])
                nc.vector.reduce_sum(ccin[:, j - t0:j - t0 + 1], acc4,
                                     axis=mybir.AxisListType.X)
            # masked-column correction (same count every tile), then gather
            nc.vector.tensor_scalar_add(ccin, ccin, mneg_sb)
            ccin_d = dram.tile([128, G], F32, tag="ccin_d", bufs=2)
            nc.scalar.dma_start(out=ccin_d, in_=ccin)
            ccout = dram.tile([NCORES * 128 * G], F32, tag="ccout", bufs=2)
            nc.gpsimd.collective_compute(
                "AllGather", mybir.AluOpType.bypass,
                replica_groups=[list(range(NCORES))],
                ins=[ccin_d.opt()], outs=[ccout.opt()],
            )
            for args in pending:
                emit_scales(*args)
            pending = [(t0, G, ccout)]
            if gi == 2:
                emit_copy_path()
        for args in pending:
            emit_scales(*args)

    nc.compile()
    return nc


def _get_nc(use_bgen: bool):
    if use_bgen not in _NC_CACHE:
        _NC_CACHE[use_bgen] = _build(use_bgen)
    return _NC_CACHE[use_bgen]


def kernel(hidden, attn, src_map, W_gen, b_gen, W_copy, b_copy):
    global LAST_RESULTS
    hidden = np.asarray(hidden, dtype=np.float32)
    attn = np.asarray(attn, dtype=np.float32)
    src_map = np.asarray(src_map, dtype=np.float32)
    W_gen = np.asarray(W_gen, dtype=np.float32)
    b_gen = np.asarray(b_gen, dtype=np.float32)
    W_copy = np.asarray(W_copy, dtype=np.float32)
    b_copy = np.asarray(b_copy, dtype=np.float32)

    use_bgen = bool(np.any(b_gen))
    nc = _get_nc(use_bgen)

    # p_copy on host: 2 MFLOP per-row scalar gate
    z = hidden @ W_copy[0] + float(b_copy.reshape(-1)[0])
    pc = 1.0 / (1.0 + np.exp(-z.astype(np.float64)))       # [N]
    ompc = np.ascontiguousarray(
        (1.0 - pc).astype(np.float32).reshape(NT, 128).T)  # [128, NT]
    pce = np.ascontiguousarray(np.broadcast_to(
        pc.astype(np.float32).reshape(TLEN, BATCH, 1),
        (TLEN, BATCH, CVOCAB))).reshape(TLEN, BATCH * CVOCAB)

    # hidden^T, tiled: ht[p, d*N + n] = hidden[n, d*128 + p]
    ht = np.ascontiguousarray(
        hidden.reshape(N, DT, 128).transpose(2, 1, 0)).reshape(128, DT * N)
    ht = ht.astype(FP8)

    # padded W with masked rows zeroed (PAD row + vocab padding), x64 for fp8
    Wp = np.zeros((VPAD, D), dtype=np.float32)
    Wp[:VOCAB] = W_gen
    Wp[PAD_IDX] = 0.0
    WT_all = np.ascontiguousarray(
        (Wp * WSCALE).reshape(VPAD, DT, 128).transpose(2, 1, 0)).astype(FP8)
    # WT_all[p, d, v]; per-core slice along v
    if use_bgen:
        bgp = np.zeros((VPAD,), dtype=np.float32)
        bgp[:VOCAB] = b_gen
        bgp[PAD_IDX] = 0.0
        bgp *= WSCALE

    # attn rearranged to [s, b, t]
    attn_r = np.ascontiguousarray(
        attn.reshape(TLEN, BATCH, SLEN).transpose(2, 1, 0)
    ).reshape(128, BATCH * TLEN).astype(BF16)
    smap = np.ascontiguousarray(
        src_map.reshape(SLEN, BATCH * CVOCAB)).astype(BF16)

    masked = np.zeros(VPAD, dtype=bool)
    masked[PAD_IDX] = True
    masked[VOCAB:] = True

    in_maps = []
    for c in range(NCORES):
        wt_c = np.ascontiguousarray(
            WT_all[:, :, c * VS:(c + 1) * VS]).reshape(128, DT * VS)
        mcount = int(masked[c * VS:(c + 1) * VS].sum())
        m = {
            "wt": wt_c,
            "ht": ht,
            "attn_r": attn_r,
            "smap": smap,
            "pce": pce,
            "ompc": ompc,
            "mneg": np.array([[-float(mcount)]], dtype=np.float32),
        }
        if use_bgen:
            m["bg"] = bgp[c * VS:(c + 1) * VS].reshape(1, VS).astype(BF16)
        in_maps.append(m)

    res = run_bass_kernel_spmd(nc, in_maps, core_ids=list(range(NCORES)))
    LAST_RESULTS = res

    out = np.empty((N, VOCAB + CVOCAB), dtype=np.float32)
    for c in range(NCORES):
        lo = c * VS
        hi = min(lo + VS, VOCAB)
        if hi > lo:
            out[:, lo:hi] = res.results[c]["out_main"][:, :hi - lo].astype(
                np.float32)
    out[:, PAD_IDX] = 0.0
    out[:, VOCAB:] = res.results[0]["out_copy"]
    return out


if __name__ == "__main__":
    # build-only smoke test
    nc = _get_nc(False)
    print("build OK:", nc)


# revision 21
# speedup vs baseline: 1.0159x; 1.0159x over previous
"""CopyGenerator on 8 TRN2 NeuronCores.

Strategy: tensor-parallel split of the 50257-wide generator vocab across the
8 cores (6400 padded columns each).  Each core:
  - holds its W_gen shard (fp8 e4m3, host-scaled by 64 and pre-transposed)
    and hidden^T (fp8) resident in SBUF,
  - computes logits*64 = hidden @ (64*W_shard).T with fp8 DoubleRow matmuls
    (256-deep contraction per instruction, fp32 PSUM accum),
  - applies exp(psum/64) on the Scalar engine over 2048-wide PSUM
    superchunks (accum_out gives the row partial sums),
  - all-gathers softmax partial denominators across cores in batches of
    2-4 row tiles (5 collectives total, overlapped with later tiles),
  - scales exp by (1 - p_copy)/denom on the Vector engine (bf16 4x mode)
    and writes its bf16 output shard,
  - computes the (tiny) copy-attention path redundantly at the end.
PAD column and vocab-padding columns are handled by zeroing those W rows on
the host (=> logit 0, exp 1) and subtracting the per-core masked-column
count from the partial denominator; the host zeroes the PAD output column.
p_copy = sigmoid(hidden @ W_copy.T + b_copy) is a 2 MFLOP per-row scalar;
it is computed host-side in f32 and fed in as two small tensors.

kernel(**inputs) takes the full unsharded inputs and returns the full
[2048, 50321] float32 output.
"""

import os
import sys

for _p in ("/opt/trn_rl_repo", "/opt/trn_rl_repo/concourse"):
    if _p not in sys.path:
        sys.path.insert(0, _p)

from contextlib import ExitStack

import ml_dtypes
import numpy as np

import concourse.bass as bass
import concourse.mybir as mybir
import concourse.tile as tile
from concourse import bacc
from concourse.bass_utils import run_bass_kernel_spmd

# ---- problem constants (hardcoded per the self-contained-kernel contract) ----
N, D = 2048, 1024                 # tlen*batch rows, hidden dim
TLEN, BATCH, SLEN, CVOCAB = 64, 32, 128, 64
VOCAB = 50257
PAD_IDX = 0
NCORES = 8
VS = 6400                         # per-core padded vocab shard width
VPAD = VS * NCORES                # 51200
DT = D // 128                     # 8 contraction tiles
NT = N // 128                     # 16 row tiles
WSCALE = 64.0                     # host pre-scale on W (fp8 subnormal escape)

# matmul/exp chunks: [(col offset, width)]; each is one 2-bank PSUM tile
# (bufs=4 -> ring depth 4, so a chunk's matmuls never wait on its own
# activation draining -- ~4.6us of slack per chunk)
SC = [(q, 1024) for q in range(0, 6144, 1024)] + [(6144, 256)]
# scale/store chunks (read exp from SBUF; wider to amortize DVE/DMA fixed
# costs -- independent of the PSUM chunking above)
SCOUT = [(0, 2048), (2048, 2048), (4096, 2048), (6144, 256)]
# denominator all-gather batching: [(first tile, n tiles)]; first group is
# small so the first collective (15us fixed latency) completes early, and the
# last two are single tiles so only one tile's scale+store sits in the tail.
GROUPS = [(0, 2), (2, 4), (6, 4), (10, 4), (14, 2)]

BF16 = ml_dtypes.bfloat16
FP8 = ml_dtypes.float8_e4m3
F32 = mybir.dt.float32
BF16_T = mybir.dt.bfloat16
FP8_T = mybir.dt.float8e4
DR = mybir.MatmulPerfMode.DoubleRow

LAST_RESULTS = None               # BassKernelResults of the most recent run
_NC_CACHE = {}


def _build(use_bgen: bool):
    nc = bacc.Bacc("TRN2", target_bir_lowering=False, debug=False,
                   num_devices=NCORES)

    wt = nc.dram_tensor("wt", [128, DT * VS], FP8_T, kind="ExternalInput").ap()
    ht = nc.dram_tensor("ht", [128, DT * N], FP8_T, kind="ExternalInput").ap()
    attn_r = nc.dram_tensor("attn_r", [128, BATCH * TLEN], BF16_T,
                            kind="ExternalInput").ap()
    smap = nc.dram_tensor("smap", [128, BATCH * CVOCAB], BF16_T,
                          kind="ExternalInput").ap()
    pce = nc.dram_tensor("pce", [TLEN, BATCH * CVOCAB], F32,
                         kind="ExternalInput").ap()
    ompc = nc.dram_tensor("ompc", [128, NT], F32, kind="ExternalInput").ap()
    mneg = nc.dram_tensor("mneg", [1, 1], F32, kind="ExternalInput").ap()
    if use_bgen:
        bg = nc.dram_tensor("bg", [1, VS], BF16_T, kind="ExternalInput").ap()
    out_main = nc.dram_tensor("out_main", [N, VS], BF16_T,
                              kind="ExternalOutput").ap()
    out_copy = nc.dram_tensor("out_copy", [N, CVOCAB], F32,
                              kind="ExternalOutput").ap()

    with tile.TileContext(nc) as tc, ExitStack() as ctx:
        singles = ctx.enter_context(tc.tile_pool(name="singles", bufs=1))
        dram = ctx.enter_context(tc.tile_pool(name="dram", bufs=1, space="DRAM"))

        # ---- resident inputs ----
        # interleave hidden^T and first-superchunk W by dp-pair so tile 0's
        # first matmuls start after ~3us of DMA instead of the full load
        ht_sb = singles.tile([128, DT, N], FP8_T)
        wt_sb = singles.tile([128, DT, VS], FP8_T)
        ht3 = ht.rearrange("p (d n) -> p d n", d=DT)
        for dp in range(DT // 2):
            nc.sync.dma_start(out=ht_sb[:, 2 * dp:2 * dp + 2, :],
                              in_=ht3[:, 2 * dp:2 * dp + 2, :])
            for d in (2 * dp, 2 * dp + 1):
                nc.gpsimd.dma_start(out=wt_sb[:, d, 0:1024],
                                    in_=wt[:, d * VS:d * VS + 1024])
        for lo, hi in ((1024, 3072), (3072, 6400)):
            for d in range(DT):
                nc.gpsimd.dma_start(out=wt_sb[:, d, lo:hi],
                                    in_=wt[:, d * VS + lo:d * VS + hi])
        ompc_sb = singles.tile([128, NT], F32)
        nc.sync.dma_start(out=ompc_sb, in_=ompc)
        mneg_sb = singles.tile([128, 1], F32)
        nc.sync.dma_start(out=mneg_sb, in_=mneg.to_broadcast((128, 1)))
        # copy-path inputs are only needed at the end; keep them last in queue
        attn_sb = singles.tile([128, BATCH * TLEN], BF16_T)
        nc.gpsimd.dma_start(out=attn_sb, in_=attn_r)
        sm_sb = singles.tile([128, BATCH * CVOCAB], BF16_T)
        nc.gpsimd.dma_start(out=sm_sb, in_=smap)
        pce_sb = singles.tile([TLEN, BATCH * CVOCAB], F32)
        nc.gpsimd.dma_start(out=pce_sb, in_=pce)
        if use_bgen:
            bg_sb = singles.tile([1, VS], BF16_T)
            nc.sync.dma_start(out=bg_sb, in_=bg)
            ones_sb = singles.tile([1, N], BF16_T)
            nc.vector.memset(ones_sb, 1.0)

        expp = ctx.enter_context(tc.tile_pool(name="expp", bufs=6))
        accp = ctx.enter_context(tc.tile_pool(name="accp", bufs=3))
        ccp = ctx.enter_context(tc.tile_pool(name="ccp", bufs=2))
        smallp = ctx.enter_context(tc.tile_pool(name="small", bufs=2))
        ostp = ctx.enter_context(tc.tile_pool(name="ostp", bufs=4))
        psp = ctx.enter_context(tc.tile_pool(name="ps", bufs=4, space="PSUM"))

        # ---- main loop: 16 row tiles in 5 denominator groups ----
        # The scale/store block for group k-1 is emitted AFTER group k's
        # collective is issued: the in-order Vector queue then never blocks
        # on a collective that hasn't had a full group of compute to hide in.
        exps = {}
        pending = []                  # [(t0, G, ccout)] awaiting scale block

        def emit_scales(t0, G, ccout):
            parts = smallp.tile([128, G, NCORES], F32, tag="parts",
                                padded_shape=[128, 4, NCORES])
            nc.scalar.dma_start(
                out=parts,
                in_=ccout.rearrange("(r p g) -> p g r", p=128, g=G))
            den = smallp.tile([128, G], F32, tag="den",
                              padded_shape=[128, 4])
            nc.vector.reduce_sum(den, parts, axis=mybir.AxisListType.X)
            rden = smallp.tile([128, G], F32, tag="rden",
                               padded_shape=[128, 4])
            nc.vector.reciprocal(rden, den)
            fs = smallp.tile([128, G], F32, tag="fs", padded_shape=[128, 4])
            nc.vector.tensor_mul(fs, rden, ompc_sb[:, t0:t0 + G])
            for j in range(t0, t0 + G):
                for c0, cw in SCOUT:
                    ost = ostp.tile([128, cw], BF16_T, tag="ost",
                                    padded_shape=[128, 2048])
                    nc.vector.tensor_scalar_mul(ost, exps[j][:, c0:c0 + cw],
                                                fs[:, j - t0:j - t0 + 1])
                    nc.sync.dma_start(
                        out=out_main[j * 128:(j + 1) * 128, c0:c0 + cw],
                        in_=ost)
                del exps[j]

        def emit_copy_path():
            # per-batch [64t,128s] @ [128s,64c], x p_copy; tiny -- emitted
            # mid-kernel so none of its work lands in the tail
            oc_flat = out_copy.rearrange("(t b) c -> t (b c)", b=BATCH)
            BB = 8                               # batches per psum tile
            for g in range(BATCH // BB):
                cp = psp.tile([TLEN, BB * CVOCAB], F32, tag="psm",
                              padded_shape=[128, 1024])
                for bb in range(BB):
                    b = g * BB + bb
                    nc.tensor.matmul(
                        cp[:, bb * CVOCAB:(bb + 1) * CVOCAB],
                        lhsT=attn_sb[:, b * TLEN:(b + 1) * TLEN],
                        rhs=sm_sb[:, b * CVOCAB:(b + 1) * CVOCAB],
                        start=True, stop=True,
                    )
                oc = ostp.tile([TLEN, BB * CVOCAB], F32, tag="oc", bufs=2)
                nc.vector.tensor_mul(
                    oc, cp, pce_sb[:, g * BB * CVOCAB:(g + 1) * BB * CVOCAB])
                nc.sync.dma_start(
                    out=oc_flat[:, g * BB * CVOCAB:(g + 1) * BB * CVOCAB],
                    in_=oc)

        for gi, (t0, G) in enumerate(GROUPS):
            ccin = ccp.tile([128, G], F32, tag="ccin",
                            padded_shape=[128, 4])
            for j in range(t0, t0 + G):
                n0 = j * 128
                exp_sb = expp.tile([128, VS], BF16_T, tag="exp")
                exps[j] = exp_sb
                acc4 = accp.tile([128, len(SC)], F32, tag="acc")
                for ci, (c0, cw) in enumerate(SC):
                    psm = psp.tile([128, cw], F32, tag="psm",
                                   padded_shape=[128, 1024])
                    for dp in range(DT // 2):
                        for q in range(0, cw, 512):
                            qw = min(512, cw - q)
                            nc.tensor.matmul(
                                psm[:, q:q + qw],
                                lhsT=ht_sb[:, 2 * dp:2 * dp + 2, n0:n0 + 128],
                                rhs=wt_sb[:, 2 * dp:2 * dp + 2,
                                          c0 + q:c0 + q + qw],
                                start=(dp == 0),
                                stop=(dp == DT // 2 - 1) and not use_bgen,
                                perf_mode=DR,
                            )
                    if use_bgen:
                        nq = [q for q in range(0, cw, 512)]
                        for qi, q in enumerate(nq):
                            qw = min(512, cw - q)
                            nc.tensor.matmul(
                                psm[:, q:q + qw],
                                lhsT=ones_sb[:, n0:n0 + 128],
                                rhs=bg_sb[:, c0 + q:c0 + q + qw],
                                start=False, stop=(qi == len(nq) - 1),
                                skip_group_check=True,
                            )
                    nc.scalar.activation(exp_sb[:, c0:c0 + cw], psm[:, 0:cw],
                                         mybir.ActivationFunctionType.Exp,
                                         scale=1.0 / WSCALE,
                                         accum_out=acc4[:, ci:ci + 1])
                nc.vector.reduce_sum(ccin[:, j - t0:j - t0 + 1], acc4,
                                     axis=mybir.AxisListType.X)
            # masked-column correction (same count every tile), then gather
            nc.vector.tensor_scalar_add(ccin, ccin, mneg_sb)
            ccin_d = dram.tile([128, G], F32, tag="ccin_d", bufs=2)
            nc.scalar.dma_start(out=ccin_d, in_=ccin)
            ccout = dram.tile([NCORES * 128 * G], F32, tag="ccout", bufs=2)
            nc.gpsimd.collective_compute(
                "AllGather", mybir.AluOpType.bypass,
                replica_groups=[list(range(NCORES))],
                ins=[ccin_d.opt()], outs=[ccout.opt()],
            )
            for args in pending:
                emit_scales(*args)
            pending = [(t0, G, ccout)]
            if gi == 2:
                emit_copy_path()
        for args in pending:
            emit_scales(*args)

    nc.compile()
    return nc


def _get_nc(use_bgen: bool):
    if use_bgen not in _NC_CACHE:
        _NC_CACHE[use_bgen] = _build(use_bgen)
    return _NC_CACHE[use_bgen]


def kernel(hidden, attn, src_map, W_gen, b_gen, W_copy, b_copy):
    global LAST_RESULTS
    hidden = np.asarray(hidden, dtype=np.float32)
    attn = np.asarray(attn, dtype=np.float32)
    src_map = np.asarray(src_map, dtype=np.float32)
    W_gen = np.asarray(W_gen, dtype=np.float32)
    b_gen = np.asarray(b_gen, dtype=np.float32)
    W_copy = np.asarray(W_copy, dtype=np.float32)
    b_copy = np.asarray(b_copy, dtype=np.float32)

    use_bgen = bool(np.any(b_gen))
    nc = _get_nc(use_bgen)

    # p_copy on host: 2 MFLOP per-row scalar gate
    z = hidden @ W_copy[0] + float(b_copy.reshape(-1)[0])
    pc = 1.0 / (1.0 + np.exp(-z.astype(np.float64)))       # [N]
    ompc = np.ascontiguousarray(
        (1.0 - pc).astype(np.float32).reshape(NT, 128).T)  # [128, NT]
    pce = np.ascontiguousarray(np.broadcast_to(
        pc.astype(np.float32).reshape(TLEN, BATCH, 1),
        (TLEN, BATCH, CVOCAB))).reshape(TLEN, BATCH * CVOCAB)

    # hidden^T, tiled: ht[p, d*N + n] = hidden[n, d*128 + p]
    ht = np.ascontiguousarray(
        hidden.reshape(N, DT, 128).transpose(2, 1, 0)).reshape(128, DT * N)
    ht = ht.astype(FP8)

    # padded W with masked rows zeroed (PAD row + vocab padding), x64 for fp8
    Wp = np.zeros((VPAD, D), dtype=np.float32)
    Wp[:VOCAB] = W_gen
    Wp[PAD_IDX] = 0.0
    WT_all = np.ascontiguousarray(
        (Wp * WSCALE).reshape(VPAD, DT, 128).transpose(2, 1, 0)).astype(FP8)
    # WT_all[p, d, v]; per-core slice along v
    if use_bgen:
        bgp = np.zeros((VPAD,), dtype=np.float32)
        bgp[:VOCAB] = b_gen
        bgp[PAD_IDX] = 0.0
        bgp *= WSCALE

    # attn rearranged to [s, b, t]
    attn_r = np.ascontiguousarray(
        attn.reshape(TLEN, BATCH, SLEN).transpose(2, 1, 0)
    ).reshape(128, BATCH * TLEN).astype(BF16)
    smap = np.ascontiguousarray(
        src_map.reshape(SLEN, BATCH * CVOCAB)).astype(BF16)

    masked = np.zeros(VPAD, dtype=bool)
    masked[PAD_IDX] = True
    masked[VOCAB:] = True

    in_maps = []
    for c in range(NCORES):
        wt_c = np.ascontiguousarray(
            WT_all[:, :, c * VS:(c + 1) * VS]).reshape(128, DT * VS)
        mcount = int(masked[c * VS:(c + 1) * VS].sum())
        m = {
            "wt": wt_c,
            "ht": ht,
            "attn_r": attn_r,
            "smap": smap,
            "pce": pce,
            "ompc": ompc,
            "mneg": np.array([[-float(mcount)]], dtype=np.float32),
        }
        if use_bgen:
            m["bg"] = bgp[c * VS:(c + 1) * VS].reshape(1, VS).astype(BF16)
        in_maps.append(m)

    res = run_bass_kernel_spmd(nc, in_maps, core_ids=list(range(NCORES)))
    LAST_RESULTS = res

    out = np.empty((N, VOCAB + CVOCAB), dtype=np.float32)
    for c in range(NCORES):
        lo = c * VS
        hi = min(lo + VS, VOCAB)
        if hi > lo:
            out[:, lo:hi] = res.results[c]["out_main"][:, :hi - lo].astype(
                np.float32)
    out[:, PAD_IDX] = 0.0
    out[:, VOCAB:] = res.results[0]["out_copy"]
    return out


if __name__ == "__main__":
    # build-only smoke test
    nc = _get_nc(False)
    print("build OK:", nc)


# revision 22
# speedup vs baseline: 1.0405x; 1.0242x over previous
"""CopyGenerator on 8 TRN2 NeuronCores.

Strategy: tensor-parallel split of the 50257-wide generator vocab across the
8 cores (6400 padded columns each).  Each core:
  - holds its W_gen shard (fp8 e4m3, host-scaled by 64 and pre-transposed)
    and hidden^T (fp8) resident in SBUF,
  - computes logits*64 = hidden @ (64*W_shard).T with fp8 DoubleRow matmuls
    (256-deep contraction per instruction, fp32 PSUM accum),
  - applies exp(psum/64) on the Scalar engine over 2048-wide PSUM
    superchunks (accum_out gives the row partial sums),
  - all-gathers softmax partial denominators across cores in batches of
    2-4 row tiles (5 collectives total, overlapped with later tiles),
  - scales exp by (1 - p_copy)/denom on the Vector engine (bf16 4x mode)
    and writes its bf16 output shard,
  - computes the (tiny) copy-attention path redundantly at the end.
PAD column and vocab-padding columns are handled by zeroing those W rows on
the host (=> logit 0, exp 1) and subtracting the per-core masked-column
count from the partial denominator; the host zeroes the PAD output column.
p_copy = sigmoid(hidden @ W_copy.T + b_copy) is a 2 MFLOP per-row scalar;
it is computed host-side in f32 and fed in as two small tensors.

kernel(**inputs) takes the full unsharded inputs and returns the full
[2048, 50321] float32 output.
"""

import os
import sys

for _p in ("/opt/trn_rl_repo", "/opt/trn_rl_repo/concourse"):
    if _p not in sys.path:
        sys.path.insert(0, _p)

from contextlib import ExitStack

import ml_dtypes
import numpy as np

import concourse.bass as bass
import concourse.mybir as mybir
import concourse.tile as tile
from concourse import bacc
from concourse.bass_utils import run_bass_kernel_spmd

# ---- problem constants (hardcoded per the self-contained-kernel contract) ----
N, D = 2048, 1024                 # tlen*batch rows, hidden dim
TLEN, BATCH, SLEN, CVOCAB = 64, 32, 128, 64
VOCAB = 50257
PAD_IDX = 0
NCORES = 8
VS = 6400                         # per-core padded vocab shard width
VPAD = VS * NCORES                # 51200
DT = D // 128                     # 8 contraction tiles
NT = N // 128                     # 16 row tiles
WSCALE = 64.0                     # host pre-scale on W (fp8 subnormal escape)

# matmul/exp chunks: [(col offset, width)]; each is one 2-bank PSUM tile
# (bufs=4 -> ring depth 4, so a chunk's matmuls never wait on its own
# activation draining -- ~4.6us of slack per chunk)
SC = [(q, 1024) for q in range(0, 6144, 1024)] + [(6144, 256)]
# scale/store chunks (read exp from SBUF; wider to amortize DVE/DMA fixed
# costs -- independent of the PSUM chunking above)
SCOUT = [(0, 2048), (2048, 2048), (4096, 2048), (6144, 256)]
# denominator all-gather batching: [(first tile, n tiles)]; first group is
# small so the first collective (15us fixed latency) completes early, and the
# last two are single tiles so only one tile's scale+store sits in the tail.
GROUPS = [(0, 2), (2, 4), (6, 4), (10, 4), (14, 2)]

BF16 = ml_dtypes.bfloat16
FP8 = ml_dtypes.float8_e4m3
F32 = mybir.dt.float32
BF16_T = mybir.dt.bfloat16
FP8_T = mybir.dt.float8e4
DR = mybir.MatmulPerfMode.DoubleRow

LAST_RESULTS = None               # BassKernelResults of the most recent run
_NC_CACHE = {}


def _build(use_bgen: bool):
    nc = bacc.Bacc("TRN2", target_bir_lowering=False, debug=False,
                   num_devices=NCORES)

    wt = nc.dram_tensor("wt", [128, DT * VS], FP8_T, kind="ExternalInput").ap()
    ht = nc.dram_tensor("ht", [128, DT * N], FP8_T, kind="ExternalInput").ap()
    attn_r = nc.dram_tensor("attn_r", [128, BATCH * TLEN], BF16_T,
                            kind="ExternalInput").ap()
    smap = nc.dram_tensor("smap", [128, BATCH * CVOCAB], BF16_T,
                          kind="ExternalInput").ap()
    pce = nc.dram_tensor("pce", [TLEN, BATCH * CVOCAB], F32,
                         kind="ExternalInput").ap()
    ompc = nc.dram_tensor("ompc", [128, NT], F32, kind="ExternalInput").ap()
    mneg = nc.dram_tensor("mneg", [1, 1], F32, kind="ExternalInput").ap()
    if use_bgen:
        bg = nc.dram_tensor("bg", [1, VS], BF16_T, kind="ExternalInput").ap()
    out_main = nc.dram_tensor("out_main", [N, VS], BF16_T,
                              kind="ExternalOutput").ap()
    out_copy = nc.dram_tensor("out_copy", [N, CVOCAB], F32,
                              kind="ExternalOutput").ap()

    with tile.TileContext(nc) as tc, ExitStack() as ctx:
        singles = ctx.enter_context(tc.tile_pool(name="singles", bufs=1))
        dram = ctx.enter_context(tc.tile_pool(name="dram", bufs=1, space="DRAM"))

        # ---- resident inputs ----
        # interleave hidden^T and first-superchunk W by dp-pair so tile 0's
        # first matmuls start after ~3us of DMA instead of the full load
        ht_sb = singles.tile([128, DT, N], FP8_T)
        wt_sb = singles.tile([128, DT, VS], FP8_T)
        ht3 = ht.rearrange("p (d n) -> p d n", d=DT)
        for dp in range(DT // 2):
            nc.sync.dma_start(out=ht_sb[:, 2 * dp:2 * dp + 2, :],
                              in_=ht3[:, 2 * dp:2 * dp + 2, :])
            for d in (2 * dp, 2 * dp + 1):
                nc.gpsimd.dma_start(out=wt_sb[:, d, 0:1024],
                                    in_=wt[:, d * VS:d * VS + 1024])
        for lo, hi in ((1024, 3072), (3072, 6400)):
            for d in range(DT):
                nc.gpsimd.dma_start(out=wt_sb[:, d, lo:hi],
                                    in_=wt[:, d * VS + lo:d * VS + hi])
        ompc_sb = singles.tile([128, NT], F32)
        nc.sync.dma_start(out=ompc_sb, in_=ompc)
        mneg_sb = singles.tile([128, 1], F32)
        nc.sync.dma_start(out=mneg_sb, in_=mneg.to_broadcast((128, 1)))
        # copy-path inputs are only needed at the end; keep them last in queue
        attn_sb = singles.tile([128, BATCH * TLEN], BF16_T)
        nc.gpsimd.dma_start(out=attn_sb, in_=attn_r)
        sm_sb = singles.tile([128, BATCH * CVOCAB], BF16_T)
        nc.gpsimd.dma_start(out=sm_sb, in_=smap)
        pce_sb = singles.tile([TLEN, BATCH * CVOCAB], F32)
        nc.gpsimd.dma_start(out=pce_sb, in_=pce)
        if use_bgen:
            bg_sb = singles.tile([1, VS], BF16_T)
            nc.sync.dma_start(out=bg_sb, in_=bg)
            ones_sb = singles.tile([1, N], BF16_T)
            nc.vector.memset(ones_sb, 1.0)

        expp = ctx.enter_context(tc.tile_pool(name="expp", bufs=6))
        accp = ctx.enter_context(tc.tile_pool(name="accp", bufs=3))
        ccp = ctx.enter_context(tc.tile_pool(name="ccp", bufs=2))
        smallp = ctx.enter_context(tc.tile_pool(name="small", bufs=2))
        ostp = ctx.enter_context(tc.tile_pool(name="ostp", bufs=4))
        psp = ctx.enter_context(tc.tile_pool(name="ps", bufs=4, space="PSUM"))

        # ---- main loop: 16 row tiles in 5 denominator groups ----
        # The scale/store block for group k-1 is emitted AFTER group k's
        # collective is issued: the in-order Vector queue then never blocks
        # on a collective that hasn't had a full group of compute to hide in.
        exps = {}
        pending = []                  # [(t0, G, ccout)] awaiting scale block

        def emit_scales(t0, G, ccout):
            parts = smallp.tile([128, G, NCORES], F32, tag="parts",
                                padded_shape=[128, 4, NCORES])
            nc.scalar.dma_start(
                out=parts,
                in_=ccout.rearrange("(r p g) -> p g r", p=128, g=G))
            den = smallp.tile([128, G], F32, tag="den",
                              padded_shape=[128, 4])
            nc.vector.reduce_sum(den, parts, axis=mybir.AxisListType.X)
            rden = smallp.tile([128, G], F32, tag="rden",
                               padded_shape=[128, 4])
            nc.vector.reciprocal(rden, den)
            fs = smallp.tile([128, G], F32, tag="fs", padded_shape=[128, 4])
            nc.vector.tensor_mul(fs, rden, ompc_sb[:, t0:t0 + G])
            for j in range(t0, t0 + G):
                for c0, cw in SCOUT:
                    ost = ostp.tile([128, cw], BF16_T, tag="ost",
                                    padded_shape=[128, 2048])
                    nc.vector.tensor_scalar_mul(ost, exps[j][:, c0:c0 + cw],
                                                fs[:, j - t0:j - t0 + 1])
                    nc.sync.dma_start(
                        out=out_main[j * 128:(j + 1) * 128, c0:c0 + cw],
                        in_=ost)
                del exps[j]

        def emit_copy_path():
            # per-batch [64t,128s] @ [128s,64c], x p_copy; tiny -- emitted
            # mid-kernel so none of its work lands in the tail
            oc_flat = out_copy.rearrange("(t b) c -> t (b c)", b=BATCH)
            BB = 8                               # batches per psum tile
            for g in range(BATCH // BB):
                cp = psp.tile([TLEN, BB * CVOCAB], F32, tag="psm",
                              padded_shape=[128, 1024])
                for bb in range(BB):
                    b = g * BB + bb
                    nc.tensor.matmul(
                        cp[:, bb * CVOCAB:(bb + 1) * CVOCAB],
                        lhsT=attn_sb[:, b * TLEN:(b + 1) * TLEN],
                        rhs=sm_sb[:, b * CVOCAB:(b + 1) * CVOCAB],
                        start=True, stop=True,
                    )
                oc = ostp.tile([TLEN, BB * CVOCAB], F32, tag="oc", bufs=2)
                nc.vector.tensor_mul(
                    oc, cp, pce_sb[:, g * BB * CVOCAB:(g + 1) * BB * CVOCAB])
                nc.sync.dma_start(
                    out=oc_flat[:, g * BB * CVOCAB:(g + 1) * BB * CVOCAB],
                    in_=oc)

        for gi, (t0, G) in enumerate(GROUPS):
            ccin = ccp.tile([128, G], F32, tag="ccin",
                            padded_shape=[128, 4])
            for j in range(t0, t0 + G):
                # previous group's scale block goes after our second tile:
                # late enough that its collective has completed (no Vector
                # head-of-line block), early enough to free exp buffers
                if j == t0 + 2 and pending:
                    for args in pending:
                        emit_scales(*args)
                    pending = []
                n0 = j * 128
                exp_sb = expp.tile([128, VS], BF16_T, tag="exp")
                exps[j] = exp_sb
                acc4 = accp.tile([128, len(SC)], F32, tag="acc")
                for ci, (c0, cw) in enumerate(SC):
                    psm = psp.tile([128, cw], F32, tag="psm",
                                   padded_shape=[128, 1024])
                    for dp in range(DT // 2):
                        for q in range(0, cw, 512):
                            qw = min(512, cw - q)
                            nc.tensor.matmul(
                                psm[:, q:q + qw],
                                lhsT=ht_sb[:, 2 * dp:2 * dp + 2, n0:n0 + 128],
                                rhs=wt_sb[:, 2 * dp:2 * dp + 2,
                                          c0 + q:c0 + q + qw],
                                start=(dp == 0),
                                stop=(dp == DT // 2 - 1) and not use_bgen,
                                perf_mode=DR,
                            )
                    if use_bgen:
                        nq = [q for q in range(0, cw, 512)]
                        for qi, q in enumerate(nq):
                            qw = min(512, cw - q)
                            nc.tensor.matmul(
                                psm[:, q:q + qw],
                                lhsT=ones_sb[:, n0:n0 + 128],
                                rhs=bg_sb[:, c0 + q:c0 + q + qw],
                                start=False, stop=(qi == len(nq) - 1),
                                skip_group_check=True,
                            )
                    nc.scalar.activation(exp_sb[:, c0:c0 + cw], psm[:, 0:cw],
                                         mybir.ActivationFunctionType.Exp,
                                         scale=1.0 / WSCALE,
                                         accum_out=acc4[:, ci:ci + 1])
                nc.vector.reduce_sum(ccin[:, j - t0:j - t0 + 1], acc4,
                                     axis=mybir.AxisListType.X)
            # masked-column correction (same count every tile), then gather
            nc.vector.tensor_scalar_add(ccin, ccin, mneg_sb)
            ccin_d = dram.tile([128, G], F32, tag="ccin_d", bufs=2)
            nc.scalar.dma_start(out=ccin_d, in_=ccin)
            ccout = dram.tile([NCORES * 128 * G], F32, tag="ccout", bufs=2)
            nc.gpsimd.collective_compute(
                "AllGather", mybir.AluOpType.bypass,
                replica_groups=[list(range(NCORES))],
                ins=[ccin_d.opt()], outs=[ccout.opt()],
            )
            for args in pending:
                emit_scales(*args)
            pending = [(t0, G, ccout)]
            if gi == 2:
                emit_copy_path()
        for args in pending:
            emit_scales(*args)

    nc.compile()
    return nc


def _get_nc(use_bgen: bool):
    if use_bgen not in _NC_CACHE:
        _NC_CACHE[use_bgen] = _build(use_bgen)
    return _NC_CACHE[use_bgen]


def kernel(hidden, attn, src_map, W_gen, b_gen, W_copy, b_copy):
    global LAST_RESULTS
    hidden = np.asarray(hidden, dtype=np.float32)
    attn = np.asarray(attn, dtype=np.float32)
    src_map = np.asarray(src_map, dtype=np.float32)
    W_gen = np.asarray(W_gen, dtype=np.float32)
    b_gen = np.asarray(b_gen, dtype=np.float32)
    W_copy = np.asarray(W_copy, dtype=np.float32)
    b_copy = np.asarray(b_copy, dtype=np.float32)

    use_bgen = bool(np.any(b_gen))
    nc = _get_nc(use_bgen)

    # p_copy on host: 2 MFLOP per-row scalar gate
    z = hidden @ W_copy[0] + float(b_copy.reshape(-1)[0])
    pc = 1.0 / (1.0 + np.exp(-z.astype(np.float64)))       # [N]
    ompc = np.ascontiguousarray(
        (1.0 - pc).astype(np.float32).reshape(NT, 128).T)  # [128, NT]
    pce = np.ascontiguousarray(np.broadcast_to(
        pc.astype(np.float32).reshape(TLEN, BATCH, 1),
        (TLEN, BATCH, CVOCAB))).reshape(TLEN, BATCH * CVOCAB)

    # hidden^T, tiled: ht[p, d*N + n] = hidden[n, d*128 + p]
    ht = np.ascontiguousarray(
        hidden.reshape(N, DT, 128).transpose(2, 1, 0)).reshape(128, DT * N)
    ht = ht.astype(FP8)

    # padded W with masked rows zeroed (PAD row + vocab padding), x64 for fp8
    Wp = np.zeros((VPAD, D), dtype=np.float32)
    Wp[:VOCAB] = W_gen
    Wp[PAD_IDX] = 0.0
    WT_all = np.ascontiguousarray(
        (Wp * WSCALE).reshape(VPAD, DT, 128).transpose(2, 1, 0)).astype(FP8)
    # WT_all[p, d, v]; per-core slice along v
    if use_bgen:
        bgp = np.zeros((VPAD,), dtype=np.float32)
        bgp[:VOCAB] = b_gen
        bgp[PAD_IDX] = 0.0
        bgp *= WSCALE

    # attn rearranged to [s, b, t]
    attn_r = np.ascontiguousarray(
        attn.reshape(TLEN, BATCH, SLEN).transpose(2, 1, 0)
    ).reshape(128, BATCH * TLEN).astype(BF16)
    smap = np.ascontiguousarray(
        src_map.reshape(SLEN, BATCH * CVOCAB)).astype(BF16)

    masked = np.zeros(VPAD, dtype=bool)
    masked[PAD_IDX] = True
    masked[VOCAB:] = True

    in_maps = []
    for c in range(NCORES):
        wt_c = np.ascontiguousarray(
            WT_all[:, :, c * VS:(c + 1) * VS]).reshape(128, DT * VS)
        mcount = int(masked[c * VS:(c + 1) * VS].sum())
        m = {
            "wt": wt_c,
            "ht": ht,
            "attn_r": attn_r,
            "smap": smap,
            "pce": pce,
            "ompc": ompc,
            "mneg": np.array([[-float(mcount)]], dtype=np.float32),
        }
        if use_bgen:
            m["bg"] = bgp[c * VS:(c + 1) * VS].reshape(1, VS).astype(BF16)
        in_maps.append(m)

    res = run_bass_kernel_spmd(nc, in_maps, core_ids=list(range(NCORES)))
    LAST_RESULTS = res

    out = np.empty((N, VOCAB + CVOCAB), dtype=np.float32)
    for c in range(NCORES):
        lo = c * VS
        hi = min(lo + VS, VOCAB)
        if hi > lo:
            out[:, lo:hi] = res.results[c]["out_main"][:, :hi - lo].astype(
                np.float32)
    out[:, PAD_IDX] = 0.0
    out[:, VOCAB:] = res.results[0]["out_copy"]
    return out


if __name__ == "__main__":
    # build-only smoke test
    nc = _get_nc(False)
    print("build OK:", nc)
